# revision 10
# baseline (speedup 1.0000x reference)
"""GTConv message-passing kernel for 8 Trainium2 NeuronCores.

Reference computation:
    filt = softmax(weight, axis=0)                       # [R, C]
    vals = (filt.T[:, :, None] * edge_value).reshape(C, R*E)
    total_edge_index = edge_index.transpose(1, 0, 2).reshape(2, R*E)

Sharding: one relation r per core (R == 8 == n_cores). Core r's slice of
both outputs is contiguous: vals[:, r*E:(r+1)*E] = filt[r, :, None] *
edge_value[r], and total_edge_index[:, r*E:(r+1)*E] = edge_index[r].
So each core scales its edge_value slice by 4 scalars and passes its
edge_index slice through untouched (DRAM->DRAM DMA) — no cross-core
communication.
"""

import sys

sys.path.insert(0, "/opt/trn_rl_repo")

from contextlib import ExitStack

import numpy as np

from concourse import bacc
from concourse import bass
from concourse import mybir
from concourse.bass_utils import run_bass_kernel_spmd
from concourse.tile import TileContext

R, C, E = 8, 4, 2_000_000
P = 128
F = E // P          # 15625 elements per partition
K = 25              # chunks along the free dim
W = F // K          # 625 elements per chunk per partition
N_CORES = 8
IDX_SPLIT = 8       # number of DMA slices for the index pass-through
USE_RAW = True      # raw bacc program (manual semaphores) vs TileContext

_CACHE: dict = {}


def _build_raw(idx_words: int) -> bass.Bass:
    """Raw bacc version of _build: same dataflow, manual semaphores, no
    Tile scheduling overhead (no tail barrier / semaphore-reset storm).

    Engine roles: Sync issues all loads + the index D2D copies, Vector does
    the 4 scalar multiplies per chunk, Scalar issues the fused 4-channel
    stores. Every DMA whose completion anyone waits on gets its own
    semaphore (HW splits one DMA's +16 into 16 independent +1s, so
    counting a shared semaphore across overlapping DMAs is not safe).
    """
    nc = bacc.Bacc()
    f32 = mybir.dt.float32
    i32 = mybir.dt.int32

    scales_d = nc.declare_dram_parameter("scales", [P, C], f32, isOutput=False)
    ev_d = nc.declare_dram_parameter("ev", [K, P, W], f32, isOutput=False)
    ei_d = nc.declare_dram_parameter(
        "ei", [IDX_SPLIT, idx_words // IDX_SPLIT], i32, isOutput=False
    )
    vals_d = nc.declare_dram_parameter("vals", [C, K, P, W], f32, isOutput=True)
    eo_d = nc.declare_dram_parameter(
        "eo", [IDX_SPLIT, idx_words // IDX_SPLIT], i32, isOutput=True
    )

    N_OT = 4      # output tile slots
    IDX_DMAS = 4  # D2D copies for the index (IDX_SPLIT rows each // this)
    assert IDX_SPLIT % IDX_DMAS == 0
    rows_per_idx = IDX_SPLIT // IDX_DMAS

    with ExitStack() as ctx:
        block = ctx.enter_context(nc.Block())
        sem_sc = ctx.enter_context(nc.semaphore("sem_sc"))
        sem_mul = ctx.enter_context(nc.semaphore("sem_mul"))
        sem_ev = [ctx.enter_context(nc.semaphore(f"sem_ev{k}")) for k in range(K)]
        sem_out = [ctx.enter_context(nc.semaphore(f"sem_out{k}")) for k in range(K)]
        sem_idx = [
            ctx.enter_context(nc.semaphore(f"sem_idx{i}")) for i in range(IDX_DMAS)
        ]
        scales_t = ctx.enter_context(nc.sbuf_tensor("scales_t", [P, C], f32))
        # one slot per chunk: every load can be issued up front with no
        # write-after-read pacing, so no load ever queues behind the big
        # index copies
        evts = [
            ctx.enter_context(nc.sbuf_tensor(f"evt{s}", [P, W], f32))
            for s in range(K)
        ]
        ots = [
            ctx.enter_context(nc.sbuf_tensor(f"ot{s}", [P, C * W], f32))
            for s in range(N_OT)
        ]

        @block.sync
        def _(sync: bass.BassEngine):
            sync.dma_start(out=scales_t[:], in_=scales_d[:]).then_inc(sem_sc, 16)
            for k in range(K):
                sync.dma_start(out=evts[k][:], in_=ev_d[k]).then_inc(sem_ev[k], 16)
            # index pass-through after all value loads: it has ~the whole
            # kernel of slack and must not starve the loads in the rings
            for i in range(IDX_DMAS):
                sl = slice(i * rows_per_idx, (i + 1) * rows_per_idx)
                sync.dma_start(out=eo_d[sl], in_=ei_d[sl]).then_inc(sem_idx[i], 16)
            for i in range(IDX_DMAS):
                sync.wait_ge(sem_idx[i], 16)

        @block.vector
        def _(vec: bass.BassEngine):
            vec.wait_ge(sem_sc, 16)
            for k in range(K):
                vec.wait_ge(sem_ev[k], 16)
                if k >= N_OT:
                    # WAR: slot k%N_OT still being stored by chunk k-N_OT
                    vec.wait_ge(sem_out[k - N_OT], 16)
                for c in range(C):
                    vec.tensor_scalar_mul(
                        ots[k % N_OT][:, c * W : (c + 1) * W],
                        evts[k][:],
                        scales_t[:, c : c + 1],
                    ).then_inc(sem_mul, 1)

        @block.scalar
        def _(act: bass.BassEngine):
            for k in range(K):
                act.wait_ge(sem_mul, C * (k + 1))
                act.dma_start(
                    out=vals_d[:, k].rearrange("c p w -> p c w"),
                    in_=ots[k % N_OT][:].rearrange("p (c w) -> p c w", c=C),
                ).then_inc(sem_out[k], 16)
            for k in range(K):
                act.wait_ge(sem_out[k], 16)

    nc.finalize()
    return nc


def _build(idx_words: int) -> bass.Bass:
    """Build the per-core SPMD program. idx_words = number of int32 words
    in one core's edge_index slice (depends on input itemsize)."""
    nc = bacc.Bacc()
    f32 = mybir.dt.float32
    i32 = mybir.dt.int32

    scales_d = nc.declare_dram_parameter("scales", [P, C], f32, isOutput=False)
    ev_d = nc.declare_dram_parameter("ev", [K, P, W], f32, isOutput=False)
    ei_d = nc.declare_dram_parameter(
        "ei", [IDX_SPLIT, idx_words // IDX_SPLIT], i32, isOutput=False
    )
    vals_d = nc.declare_dram_parameter("vals", [C, K, P, W], f32, isOutput=True)
    eo_d = nc.declare_dram_parameter(
        "eo", [IDX_SPLIT, idx_words // IDX_SPLIT], i32, isOutput=True
    )

    with TileContext(nc) as tc:
        with (
            tc.tile_pool(name="const", bufs=1) as cpool,
            tc.tile_pool(name="io", bufs=3) as iopool,
            tc.tile_pool(name="out", bufs=2) as opool,
        ):
            scales_t = cpool.tile([P, C], f32)
            nc.sync.dma_start(out=scales_t[:], in_=scales_d[:])

            # Scale pipeline: Sync issues chunk loads (interleaved with the
            # index D2D copies), DVE does the 4 scalar multiplies, Scalar
            # issues one fused 4-channel store per chunk. Keeping the
            # compute-dependent stores off Sync's sequencer lets the
            # independent loads/copies issue without stalling behind them.
            idx_per_iter = (IDX_SPLIT + K - 1) // K
            for k in range(K):
                evt = iopool.tile([P, W], f32, tag="evt")
                nc.sync.dma_start(out=evt[:], in_=ev_d[k])
                for i in range(k * idx_per_iter, min((k + 1) * idx_per_iter, IDX_SPLIT)):
                    nc.sync.dma_start(out=eo_d[i], in_=ei_d[i])
                ot = opool.tile([P, C * W], f32, tag="ot")
                for c in range(C):
                    nc.vector.tensor_scalar_mul(
                        ot[:, c * W : (c + 1) * W], evt[:], scales_t[:, c : c + 1]
                    )
                nc.scalar.dma_start(
                    out=vals_d[:, k].rearrange("c p w -> p c w"),
                    in_=ot[:].rearrange("p (c w) -> p c w", c=C),
                )

    nc.finalize()
    return nc


def _get_nc(idx_words: int) -> bass.Bass:
    key = (idx_words, USE_RAW)
    if key not in _CACHE:
        _CACHE[key] = (_build_raw if USE_RAW else _build)(idx_words)
    return _CACHE[key]


def _make_in_maps(weight, edge_index, edge_value):
    weight = np.ascontiguousarray(np.asarray(weight), dtype=np.float32)
    edge_value = np.asarray(edge_value)
    edge_index = np.asarray(edge_index)
    assert weight.shape == (R, C)
    assert edge_value.shape == (R, E)
    assert edge_index.shape == (R, 2, E)

    # softmax over relations (axis 0) in f32, matching the reference
    m = weight.max(axis=0, keepdims=True)
    ex = np.exp(weight - m, dtype=np.float32)
    filt = ex / ex.sum(axis=0, keepdims=True)  # [R, C]

    idx_words = edge_index[0].nbytes // 4
    in_maps = []
    for r in range(R):
        scales = np.ascontiguousarray(
            np.broadcast_to(filt[r], (P, C)), dtype=np.float32
        )
        ev = np.ascontiguousarray(edge_value[r], dtype=np.float32).reshape(K, P, W)
        ei = (
            np.ascontiguousarray(edge_index[r])
            .view(np.int32)
            .reshape(IDX_SPLIT, idx_words // IDX_SPLIT)
        )
        in_maps.append({"scales": scales, "ev": ev, "ei": ei})
    return in_maps, idx_words, edge_index.dtype


def _assemble(results, idx_dtype):
    vals = np.empty((C, R, E), dtype=np.float32)
    idx = np.empty((2, R, E), dtype=idx_dtype)
    words_per_row = np.dtype(idx_dtype).itemsize // 4 * E
    for r in range(R):
        vals[:, r, :] = results[r]["vals"].reshape(C, E)
        idx[:, r, :] = (
            results[r]["eo"].reshape(2, words_per_row).view(idx_dtype)
        )
    return idx.reshape(2, R * E), vals.reshape(C, R * E)


def kernel(weight, edge_index, edge_value):
    in_maps, idx_words, idx_dtype = _make_in_maps(weight, edge_index, edge_value)
    nc = _get_nc(idx_words)
    res = run_bass_kernel_spmd(nc, in_maps, list(range(N_CORES)))
    return _assemble(res.results, idx_dtype)


def kernel_profiled(weight, edge_index, edge_value, **kwargs):
    """Same as kernel() but with NTFF profiling; returns (outputs, BassKernelResults)."""
    in_maps, idx_words, idx_dtype = _make_in_maps(weight, edge_index, edge_value)
    nc = _get_nc(idx_words)
    res = run_bass_kernel_spmd(
        nc, in_maps, list(range(N_CORES)), trace=True, **kwargs
    )
    return _assemble(res.results, idx_dtype), res


# revision 13
# speedup vs baseline: 1.2585x; 1.2585x over previous
"""GTConv message-passing kernel for 8 Trainium2 NeuronCores.

Reference computation:
    filt = softmax(weight, axis=0)                       # [R, C]
    vals = (filt.T[:, :, None] * edge_value).reshape(C, R*E)
    total_edge_index = edge_index.transpose(1, 0, 2).reshape(2, R*E)

Sharding: one relation r per core (R == 8 == n_cores). Core r's slice of
both outputs is contiguous: vals[:, r*E:(r+1)*E] = filt[r, :, None] *
edge_value[r], and total_edge_index[:, r*E:(r+1)*E] = edge_index[r].
So each core scales its edge_value slice by 4 scalars and passes its
edge_index slice through untouched (DRAM->DRAM DMA) — no cross-core
communication.
"""

import sys

sys.path.insert(0, "/opt/trn_rl_repo")

from contextlib import ExitStack

import numpy as np

from concourse import bacc
from concourse import bass
from concourse import mybir
from concourse.bass_utils import run_bass_kernel_spmd
from concourse.tile import TileContext

R, C, E = 8, 4, 2_000_000
P = 128
F = E // P          # 15625 elements per partition
K = 5               # chunks along the free dim
W = F // K          # 3125 elements per chunk per partition (12.5 KB DMA
                    # descriptors per partition segment — smaller chunks
                    # shrink descriptors and tank DMA efficiency)
N_CORES = 8
IDX_SPLIT = 8       # number of DMA slices for the index pass-through
USE_RAW = True      # raw bacc program (manual semaphores) vs TileContext

_CACHE: dict = {}


def _build_raw(idx_words: int) -> bass.Bass:
    """Raw bacc version of _build: same dataflow, manual semaphores, no
    Tile scheduling overhead (no tail barrier / semaphore-reset storm).

    Engine roles: Sync issues all loads + the index D2D copies, Vector does
    the 4 scalar multiplies per chunk, Scalar issues the fused 4-channel
    stores. Every DMA whose completion anyone waits on gets its own
    semaphore (HW splits one DMA's +16 into 16 independent +1s, so
    counting a shared semaphore across overlapping DMAs is not safe).
    """
    nc = bacc.Bacc()
    f32 = mybir.dt.float32
    i32 = mybir.dt.int32

    scales_d = nc.declare_dram_parameter("scales", [P, C], f32, isOutput=False)
    ev_d = nc.declare_dram_parameter("ev", [K, P, W], f32, isOutput=False)
    ei_d = nc.declare_dram_parameter(
        "ei", [IDX_SPLIT, idx_words // IDX_SPLIT], i32, isOutput=False
    )
    vals_d = nc.declare_dram_parameter("vals", [C, K, P, W], f32, isOutput=True)
    eo_d = nc.declare_dram_parameter(
        "eo", [IDX_SPLIT, idx_words // IDX_SPLIT], i32, isOutput=True
    )

    N_OT = 2      # output tile slots
    IDX_DMAS = 4  # D2D copies for the index (IDX_SPLIT rows each // this)
    assert IDX_SPLIT % IDX_DMAS == 0
    rows_per_idx = IDX_SPLIT // IDX_DMAS

    with ExitStack() as ctx:
        block = ctx.enter_context(nc.Block())
        sem_sc = ctx.enter_context(nc.semaphore("sem_sc"))
        sem_mul = ctx.enter_context(nc.semaphore("sem_mul"))
        sem_ev = [ctx.enter_context(nc.semaphore(f"sem_ev{k}")) for k in range(K)]
        sem_out = [ctx.enter_context(nc.semaphore(f"sem_out{k}")) for k in range(K)]
        sem_idx = [
            ctx.enter_context(nc.semaphore(f"sem_idx{i}")) for i in range(IDX_DMAS)
        ]
        scales_t = ctx.enter_context(nc.sbuf_tensor("scales_t", [P, C], f32))
        # one slot per chunk: every load can be issued up front with no
        # write-after-read pacing, so no load ever queues behind the big
        # index copies
        evts = [
            ctx.enter_context(nc.sbuf_tensor(f"evt{s}", [P, W], f32))
            for s in range(K)
        ]
        ots = [
            ctx.enter_context(nc.sbuf_tensor(f"ot{s}", [P, C * W], f32))
            for s in range(N_OT)
        ]

        @block.sync
        def _(sync: bass.BassEngine):
            sync.dma_start(out=scales_t[:], in_=scales_d[:]).then_inc(sem_sc, 16)
            for k in range(K):
                sync.dma_start(out=evts[k][:], in_=ev_d[k]).then_inc(sem_ev[k], 16)
            # index pass-through after all value loads: it has ~the whole
            # kernel of slack and must not starve the loads in the rings
            for i in range(IDX_DMAS):
                sl = slice(i * rows_per_idx, (i + 1) * rows_per_idx)
                sync.dma_start(out=eo_d[sl], in_=ei_d[sl]).then_inc(sem_idx[i], 16)
            for i in range(IDX_DMAS):
                sync.wait_ge(sem_idx[i], 16)

        @block.vector
        def _(vec: bass.BassEngine):
            vec.wait_ge(sem_sc, 16)
            for k in range(K):
                vec.wait_ge(sem_ev[k], 16)
                if k >= N_OT:
                    # WAR: slot k%N_OT still being stored by chunk k-N_OT
                    vec.wait_ge(sem_out[k - N_OT], 16 * C)
                for c in range(C):
                    vec.tensor_scalar_mul(
                        ots[k % N_OT][:, c * W : (c + 1) * W],
                        evts[k][:],
                        scales_t[:, c : c + 1],
                    ).then_inc(sem_mul, 1)

        @block.scalar
        def _(act: bass.BassEngine):
            # per-channel stores: each starts as soon as its multiply is
            # done, and the final store is 1.6 MB instead of 6.4 MB, so the
            # un-overlapped tail after the last multiply is short
            for k in range(K):
                for c in range(C):
                    act.wait_ge(sem_mul, C * k + c + 1)
                    act.dma_start(
                        out=vals_d[c, k],
                        in_=ots[k % N_OT][:, c * W : (c + 1) * W],
                    ).then_inc(sem_out[k], 16)
            for k in range(K):
                act.wait_ge(sem_out[k], 16 * C)

    nc.finalize()
    return nc


def _build(idx_words: int) -> bass.Bass:
    """Build the per-core SPMD program. idx_words = number of int32 words
    in one core's edge_index slice (depends on input itemsize)."""
    nc = bacc.Bacc()
    f32 = mybir.dt.float32
    i32 = mybir.dt.int32

    scales_d = nc.declare_dram_parameter("scales", [P, C], f32, isOutput=False)
    ev_d = nc.declare_dram_parameter("ev", [K, P, W], f32, isOutput=False)
    ei_d = nc.declare_dram_parameter(
        "ei", [IDX_SPLIT, idx_words // IDX_SPLIT], i32, isOutput=False
    )
    vals_d = nc.declare_dram_parameter("vals", [C, K, P, W], f32, isOutput=True)
    eo_d = nc.declare_dram_parameter(
        "eo", [IDX_SPLIT, idx_words // IDX_SPLIT], i32, isOutput=True
    )

    with TileContext(nc) as tc:
        with (
            tc.tile_pool(name="const", bufs=1) as cpool,
            tc.tile_pool(name="io", bufs=3) as iopool,
            tc.tile_pool(name="out", bufs=2) as opool,
        ):
            scales_t = cpool.tile([P, C], f32)
            nc.sync.dma_start(out=scales_t[:], in_=scales_d[:])

            # Scale pipeline: Sync issues chunk loads (interleaved with the
            # index D2D copies), DVE does the 4 scalar multiplies, Scalar
            # issues one fused 4-channel store per chunk. Keeping the
            # compute-dependent stores off Sync's sequencer lets the
            # independent loads/copies issue without stalling behind them.
            idx_per_iter = (IDX_SPLIT + K - 1) // K
            for k in range(K):
                evt = iopool.tile([P, W], f32, tag="evt")
                nc.sync.dma_start(out=evt[:], in_=ev_d[k])
                for i in range(k * idx_per_iter, min((k + 1) * idx_per_iter, IDX_SPLIT)):
                    nc.sync.dma_start(out=eo_d[i], in_=ei_d[i])
                ot = opool.tile([P, C * W], f32, tag="ot")
                for c in range(C):
                    nc.vector.tensor_scalar_mul(
                        ot[:, c * W : (c + 1) * W], evt[:], scales_t[:, c : c + 1]
                    )
                nc.scalar.dma_start(
                    out=vals_d[:, k].rearrange("c p w -> p c w"),
                    in_=ot[:].rearrange("p (c w) -> p c w", c=C),
                )

    nc.finalize()
    return nc


def _get_nc(idx_words: int) -> bass.Bass:
    key = (idx_words, USE_RAW)
    if key not in _CACHE:
        _CACHE[key] = (_build_raw if USE_RAW else _build)(idx_words)
    return _CACHE[key]


def _make_in_maps(weight, edge_index, edge_value):
    weight = np.ascontiguousarray(np.asarray(weight), dtype=np.float32)
    edge_value = np.asarray(edge_value)
    edge_index = np.asarray(edge_index)
    assert weight.shape == (R, C)
    assert edge_value.shape == (R, E)
    assert edge_index.shape == (R, 2, E)

    # softmax over relations (axis 0) in f32, matching the reference
    m = weight.max(axis=0, keepdims=True)
    ex = np.exp(weight - m, dtype=np.float32)
    filt = ex / ex.sum(axis=0, keepdims=True)  # [R, C]

    idx_words = edge_index[0].nbytes // 4
    in_maps = []
    for r in range(R):
        scales = np.ascontiguousarray(
            np.broadcast_to(filt[r], (P, C)), dtype=np.float32
        )
        ev = np.ascontiguousarray(edge_value[r], dtype=np.float32).reshape(K, P, W)
        ei = (
            np.ascontiguousarray(edge_index[r])
            .view(np.int32)
            .reshape(IDX_SPLIT, idx_words // IDX_SPLIT)
        )
        in_maps.append({"scales": scales, "ev": ev, "ei": ei})
    return in_maps, idx_words, edge_index.dtype


def _assemble(results, idx_dtype):
    vals = np.empty((C, R, E), dtype=np.float32)
    idx = np.empty((2, R, E), dtype=idx_dtype)
    words_per_row = np.dtype(idx_dtype).itemsize // 4 * E
    for r in range(R):
        vals[:, r, :] = results[r]["vals"].reshape(C, E)
        idx[:, r, :] = (
            results[r]["eo"].reshape(2, words_per_row).view(idx_dtype)
        )
    return idx.reshape(2, R * E), vals.reshape(C, R * E)


def kernel(weight, edge_index, edge_value):
    in_maps, idx_words, idx_dtype = _make_in_maps(weight, edge_index, edge_value)
    nc = _get_nc(idx_words)
    res = run_bass_kernel_spmd(nc, in_maps, list(range(N_CORES)))
    return _assemble(res.results, idx_dtype)


def kernel_profiled(weight, edge_index, edge_value, **kwargs):
    """Same as kernel() but with NTFF profiling; returns (outputs, BassKernelResults)."""
    in_maps, idx_words, idx_dtype = _make_in_maps(weight, edge_index, edge_value)
    nc = _get_nc(idx_words)
    res = run_bass_kernel_spmd(
        nc, in_maps, list(range(N_CORES)), trace=True, **kwargs
    )
    return _assemble(res.results, idx_dtype), res
